# revision 53
# baseline (speedup 1.0000x reference)
"""Trainium2 Bass kernel for BinarizeConv2dSDP.

Math (reference):
    s   = M + rv @ Z          (the rsqrt normalization is sign-preserving:
                               w = (m + rv@z) * rsqrt(...) with rsqrt > 0,
                               so sign(w) == sign(s))
    bw  = sign(s)             (O, I, 3, 3)
    ba  = sign(x)             (B, C, H, W)
    out = conv2d(ba, bw, stride 1, pad 1) * Alpha

Strategy (69.5us baseline -> ~59-61us; HW exec varies +-1us run-to-run and
the device occasionally sits in a ~68us slow regime for identical code):
    - Data-parallel over batch: 8 cores x 4 images each. M/Z/Alpha replicated.
    - Head pipeline: weights-first DMA order on the sync ring
      [M, z0-z4, alpha, x0 strips, x1 strips]; x2/x3 issue from
      the conv loop. Weight chain s = M + sum_k rv_k Z_k runs k-major on the
      DVE over per-ky chunk tiles (whole-tile dep granularity), consuming
      each z_k as it lands; per chunk: ACT sign -> 3 full [128x128] PE
      transposes -> packed-psum copy. x0/x1+ arrive as row strips in
      separate tiles so each strip's sign starts as it lands; pd deps are
      subtile-level, so conv tile t needs only the strips covering its
      rows (x0 signs are emitted AFTER the weight signs -- the static
      scheduler once ordered x0s0's sign before c2's, delaying the pack
      that gates the conv's horizontal-pair/single passes).
    - Binarized conv: sign(x) stored fp8e4 in a zero-padded [128, 58 x 64]
      SBUF image (row stride 64). Per output row-block, 5 PE passes:
      3 vertical-pair DoubleRow matmuls (K=256, pair step 64B), 1
      horizontal-pair DoubleRow for taps (2,0)+(2,1) (pair step 1B!), 1
      single matmul for tap (2,2). +-1 is exact in fp8e4/bf16; PSUM f32.
      Measured ~194ns per N=448 matmul, gapless.
    - Evac applies Alpha and writes float16 (conv values are integers
      <= 1152, exact in f16; only the Alpha scale rounds, ~2^-12); stores
      ride scalar/gpsimd queues. Host casts back to f32. The very last
      tile's evac is split across DVE+ACT into two ev tiles and stored as
      two half-row-blocks on both rings to shorten the end-of-kernel
      serial tail (ACT's activation Copy takes Alpha as a per-partition
      scale vector).
    - Measured dead ends kept off: in-channel-sharded synthesis + HBM
      AllGather (USE_AG) stalls ~77us on the collective; cross-ring DMA
      (x0 or z halves on the scalar ring) costs 1-4us; alpha issued first
      delays the z stream; alpha issued last raced the first evac once.
"""

import os
import numpy as np

import concourse.bass as bass
import concourse.tile as tile
from concourse import bacc, mybir
from concourse.bass_utils import run_bass_kernel_spmd
from concourse.masks import make_identity

F32 = mybir.dt.float32
F16 = mybir.dt.float16
BF16 = mybir.dt.bfloat16
FP8 = mybir.dt.float8e4

USE_FP8 = bool(int(os.environ.get("BASS_KERNEL_FP8", "1")))
# 5-pass conv (horizontal DoubleRow pair with 1-byte pair stride). If HW
# rejects the 1B pair offset, set to 0 for the 6-pass fallback.
USE_HPAIR = bool(int(os.environ.get("BASS_KERNEL_HPAIR", "1")))
# Shard the weight synthesis by in-channel across the 8 cores and AllGather
# the packed fp8 lhsT (18KB/core) via HBM. Measured DISASTER on this
# runtime: the collective stalls the conv ~77us (rendezvous/launch
# overhead), 130us total vs 60us without. Kept for reference, default off.
USE_AG = USE_FP8 and bool(int(os.environ.get("BASS_KERNEL_AG", "0")))

B_FULL = 32
N_CORES = 8
B_CORE = B_FULL // N_CORES  # 4 images per core
C = 128      # in channels
O = 128      # out channels
H = W = 56
HP = 58                      # padded rows
WP = 64 if USE_FP8 else 58   # padded row stride
KS = 3
NTAPS = KS * KS
IKK = C * NTAPS  # 1152
ROWS_PER_TILE = 8           # output rows per PSUM tile -> N = 8*56 = 448
N_TILE = ROWS_PER_TILE * W  # 448 fp32 <= 512 (one PSUM bank)
N_ROW_TILES = H // ROWS_PER_TILE  # 7
ADT = FP8 if USE_FP8 else BF16

# Weight-chain chunks are TAP-major (one kernel row ky per chunk, all 128
# channels) so each chunk's transposes are 3 full [128,128] PE transposes
# instead of 9 narrow ones. Pool (gpsimd) rejects InstTensorScalarPtr on
# TRN2, so the whole chain runs on the DVE: 3 chunk-ops/k (~1.4us) matches
# the ~1.7us per-z DMA cadence.
NCHUNK = KS                  # chunk g covers taps ky==g (384 elems/partition)
# x0 row strips: pd deps are subtile-level, so conv tile t only needs the
# strips covering its rows; tile0 reads x rows 0..8, so a 10-row first
# strip covers it and lands/signs sooner
X0_STRIPS = (10, 16, 16, 14)
C_SH = C // N_CORES          # in-channels synthesized per core under AG


def build_program(rv: np.ndarray, n_img: int = B_CORE):
    """Build the per-core Bass program. rv values are baked as immediates."""
    nc = bacc.Bacc(
        "TRN2",
        target_bir_lowering=False,
        debug=False,
        num_devices=N_CORES,
    )

    x_t = nc.dram_tensor("x", (n_img, C, H, W), F32, kind="ExternalInput").ap()
    a_t = nc.dram_tensor("Alpha", (O, 1, 1), F32, kind="ExternalInput").ap()
    CW = C_SH if USE_AG else C  # channel width this core synthesizes
    m_t = nc.dram_tensor("M", (O, CW, KS, KS), F32, kind="ExternalInput").ap()
    z_t = nc.dram_tensor("Z", (5, O, CW, KS, KS), F32, kind="ExternalInput").ap()
    out_t = nc.dram_tensor("out", (n_img, O, H, W), F16, kind="ExternalOutput").ap()
    if USE_AG:
        bwg_in_t = nc.dram_tensor(
            "bwg_in", (C_SH, IKK), FP8, kind="Internal"
        ).ap()
        bwg_out_t = nc.dram_tensor(
            "bwg_out", (C, IKK), FP8, kind="Internal", addr_space="Shared"
        ).ap()

    rv = np.asarray(rv, dtype=np.float32).reshape(-1)
    assert rv.shape[0] == 5

    x_flat = x_t.rearrange("n c h w -> n c (h w)")

    with tile.TileContext(nc) as tc:
        with (
            tc.tile_pool(name="const", bufs=1) as const_pool,
            tc.tile_pool(name="wsyn", bufs=1) as wsyn_pool,
            tc.tile_pool(name="imgs", bufs=1) as img_pool,
            tc.tile_pool(name="xstage", bufs=1) as x_pool,
            tc.tile_pool(name="evac", bufs=8) as ev_pool,
            tc.tile_pool(name="cpsum", bufs=6, space="PSUM") as cpsum_pool,
            tc.tile_pool(name="tpsum", bufs=1, space="PSUM") as tpsum_pool,
        ):
            # ---- head DMA issue: weights first, x0 strips interleaved ----
            alpha_sb = const_pool.tile([O, 1], F32)
            GIKK = CW * NTAPS  # weight elems per partition this core owns
            m_sb = wsyn_pool.tile([O, GIKK], F32)
            nc.sync.dma_start(m_sb, m_t.rearrange("o i kh kw -> o (i kh kw)"))
            z_sbs = []

            def dma_z(k):
                z_sb = wsyn_pool.tile([O, GIKK], F32, name=f"z{k}", tag=f"z{k}")
                nc.sync.dma_start(
                    z_sb, z_t[k].rearrange("o i kh kw -> o (i kh kw)")
                )
                z_sbs.append(z_sb)

            # x0 comes in separate strip tiles so each strip's sign can start
            # as soon as that strip lands (whole-tile dep granularity).
            x0_strip = [
                x_pool.tile([C, nr * W], F32, name=f"x0s{i}", tag=f"x0s{i}")
                for i, nr in enumerate(X0_STRIPS)
            ]
            x0_r0 = [sum(X0_STRIPS[:i]) for i in range(len(X0_STRIPS))]

            def dma_x0_strip(i, eng=None):
                (eng or nc.sync).dma_start(
                    x0_strip[i],
                    x_flat[0, :, x0_r0[i] * W : (x0_r0[i] + X0_STRIPS[i]) * W],
                )

            if USE_AG:
                for k in range(5):
                    dma_z(k)
                for i in range(len(X0_STRIPS)):
                    dma_x0_strip(i)
                nc.sync.dma_start(
                    alpha_sb, a_t.rearrange("o a b -> o (a b)")
                )
            else:
                for k in range(5):
                    dma_z(k)
                # alpha here: lands ~6us before the first evac reads it
                # (late placement raced the evac once; cross-ring placement
                # costs ~1-3us)
                nc.sync.dma_start(
                    alpha_sb, a_t.rearrange("o a b -> o (a b)")
                )
                for i in range(len(X0_STRIPS)):
                    dma_x0_strip(i)
            # images 1..n-1 stream as two strips each so their signs start
            # as soon as each strip lands (pd ready ~strip-sign after land).
            # Only x1 is issued up front: a deep backlog of outstanding DMA
            # instructions slows the PE ~2x (observed), so x2/x3 issue
            # lazily from inside the conv loop.
            XI_STRIPS = (28, 28)
            xi_r0 = (0, 28)
            x_strips = {}

            def dma_image(img):
                for j, nr in enumerate(XI_STRIPS):
                    t = x_pool.tile(
                        [C, nr * W], F32, name=f"x{img}s{j}", tag=f"x{img}s{j}"
                    )
                    nc.sync.dma_start(
                        t, x_flat[img, :, xi_r0[j] * W : (xi_r0[j] + nr) * W]
                    )
                    x_strips[(img, j)] = t

            if n_img > 1:
                dma_image(1)

            def sign_image(img):
                pd3 = padded[img]
                for j, nr in enumerate(XI_STRIPS):
                    r0 = xi_r0[j]
                    nc.scalar.sign(
                        pd3[:, 1 + r0 : 1 + r0 + nr, 1 : 1 + W],
                        x_strips[(img, j)].rearrange("c (h w) -> c h w", w=W),
                    )

            identity = const_pool.tile([128, 128], BF16)
            make_identity(nc, identity)


            # ---- per-image padded sign(x) buffers (borders zeroed once) ----
            padded = []
            for img in range(n_img):
                pd = img_pool.tile(
                    [C, HP * WP], ADT, name=f"pad{img}", tag=f"pad{img}"
                )
                pd3 = pd.rearrange("p (h w) -> p h w", w=WP)
                nc.gpsimd.memset(pd3[:, 0, 0:HP], 0.0)
                nc.gpsimd.memset(pd3[:, HP - 1, 0:HP], 0.0)
                nc.gpsimd.memset(pd3[:, 1 : HP - 1, 0:1], 0.0)
                nc.gpsimd.memset(pd3[:, 1 : HP - 1, HP - 1 : HP], 0.0)
                padded.append(pd3)

            def sign_x0_strip(i):
                r0 = x0_r0[i]
                nc.scalar.sign(
                    padded[0][:, 1 + r0 : 1 + r0 + X0_STRIPS[i], 1 : 1 + W],
                    x0_strip[i].rearrange("c (h w) -> c h w", w=W),
                )

            if USE_AG:
                # ---- sharded weight synthesis: this core synthesizes its
                # C_SH in-channel slice, packs it as the final fp8 lhsT row
                # block, AllGathers via HBM, and loads the full lhsT back ----
                s_sb = wsyn_pool.tile([O, GIKK], F32)
                for k in range(5):
                    nc.vector.scalar_tensor_tensor(
                        out=s_sb,
                        in0=z_sbs[k],
                        scalar=float(rv[k]),
                        in1=m_sb if k == 0 else s_sb,
                        op0=mybir.AluOpType.mult,
                        op1=mybir.AluOpType.add,
                    )
                bwn = wsyn_pool.tile([O, GIKK], BF16)
                nc.scalar.sign(bwn, s_sb)
                bwn3 = bwn.rearrange("o (i t) -> o i t", t=NTAPS)
                tpP = tpsum_pool.tile([C_SH, KS * 2 * O], BF16)
                tpS = tpsum_pool.tile([C_SH, KS * O], BF16)
                tpP4 = tpP.rearrange("p (a b o) -> p a b o", b=2, o=O)
                tpS3 = tpS.rearrange("p (a o) -> p a o", o=O)
                for t in range(NTAPS):
                    ky, kx = divmod(t, KS)
                    dst = tpS3[:, kx, :] if ky == 2 else tpP4[:, kx, ky, :]
                    nc.tensor.transpose(dst, bwn3[:, :, t], identity)
                bw_my = wsyn_pool.tile([C_SH, IKK], FP8)
                nc.scalar.copy(bw_my[:, 0 : KS * 2 * O], tpP)
                nc.vector.tensor_copy(bw_my[:, KS * 2 * O : IKK], tpS)
                nc.sync.dma_start(bwg_in_t, bw_my)
                nc.gpsimd.collective_compute(
                    "AllGather",
                    mybir.AluOpType.bypass,
                    replica_groups=[list(range(N_CORES))],
                    ins=[bwg_in_t],
                    outs=[bwg_out_t],
                )
                bw_all = wsyn_pool.tile([C, IKK], FP8)
                nc.sync.dma_start(bw_all, bwg_out_t)
                bw_pair = bw_all[:, 0 : KS * 2 * O].rearrange(
                    "p (a b o) -> p a b o", b=2, o=O
                )
                bw_single = bw_all[:, KS * 2 * O : IKK].rearrange(
                    "p (a o) -> p a o", o=O
                )
                for i in range(len(X0_STRIPS)):
                    sign_x0_strip(i)
            else:
                # ---- full weight synthesis: s = M + sum_k rv_k Z_k, k-major
                # over per-chunk (per-ky) tiles so each z_k is consumed as it
                # lands ----
                GSZ = C * KS  # 384 elems per partition per chunk
                m3 = m_sb.rearrange("o (i t) -> o i t", t=NTAPS)
                z3s = [
                    z.rearrange("o (i t) -> o i t", t=NTAPS) for z in z_sbs
                ]
                s_c = [
                    wsyn_pool.tile([O, GSZ], F32, name=f"s{g}", tag=f"s{g}")
                    for g in range(NCHUNK)
                ]
                bw_c = [
                    wsyn_pool.tile([O, GSZ], BF16, name=f"bw{g}", tag=f"bw{g}")
                    for g in range(NCHUNK)
                ]
                for k in range(5):
                    for g in range(NCHUNK):
                        tsl = slice(g * KS, (g + 1) * KS)
                        nc.vector.scalar_tensor_tensor(
                            out=s_c[g].rearrange("o (i t) -> o i t", t=KS),
                            in0=z3s[k][:, :, tsl],
                            scalar=float(rv[k]),
                            in1=m3[:, :, tsl]
                            if k == 0
                            else s_c[g].rearrange("o (i t) -> o i t", t=KS),
                            op0=mybir.AluOpType.mult,
                            op1=mybir.AluOpType.add,
                        )

                # per-chunk: sign -> 3 full-width PE transposes -> pack copy.
                # fp8 psum layout: tpP[(kx, ky<2, o)] pairs, tpS[(kx, o)] the
                # ky=2 taps. bf16 layout: same split (6 + 3 taps).
                if USE_FP8:
                    bw_pair = wsyn_pool.tile([C, KS, 2, O], FP8)
                    bw_single = wsyn_pool.tile([C, KS, O], FP8)
                else:
                    bw_lhsT = wsyn_pool.tile([C, NTAPS, O], BF16)
                tpP = tpsum_pool.tile([128, KS * 2 * O], BF16)
                tpS = tpsum_pool.tile([128, KS * O], BF16)
                tpP4 = tpP.rearrange("p (a b o) -> p a b o", b=2, o=O)
                tpS3 = tpS.rearrange("p (a o) -> p a o", o=O)

                def emit_chunk(g):
                    ky = g
                    nc.scalar.sign(bw_c[g], s_c[g])
                    bw3 = bw_c[g].rearrange("o (i t) -> o i t", t=KS)
                    for kx in range(KS):
                        dst = tpS3[:, kx, :] if ky == 2 else tpP4[:, kx, ky, :]
                        nc.tensor.transpose(dst, bw3[:, :, kx], identity)

                def pack_chunk(g):
                    ky = g
                    if USE_FP8:
                        dst = (
                            bw_single.rearrange("p a o -> p (a o)")
                            if ky == 2
                            else bw_pair[:, :, ky, :]
                        )
                    else:
                        dst = bw_lhsT.rearrange("p (a t) o -> p a t o", a=KS)[
                            :, ky, :, :
                        ]
                    src = tpS if ky == 2 else tpP4[:, :, ky, :]
                    nc.vector.tensor_copy(dst, src)

                emit_chunk(0)
                emit_chunk(1)
                pack_chunk(0)
                emit_chunk(2)
                pack_chunk(1)
                pack_chunk(2)

                # x0 signs emitted after the weight path: the static
                # scheduler once ordered x0s0's sign before c2's, delaying
                # the pack that gates the conv's last two passes
                for i in range(len(X0_STRIPS)):
                    sign_x0_strip(i)

            # ---- main conv loop; next image's sign emitted before this
            # image's tiles so ACT never head-of-line blocks the sign ----
            def pair_ap(win, pair_stride):
                return bass.AP(
                    win.tensor,
                    win.offset,
                    [list(win.ap[0]), [pair_stride, 2]]
                    + [list(p) for p in win.ap[1:]],
                )

            for img in range(n_img):
                if img + 2 < n_img:
                    dma_image(img + 2)
                if img + 1 < n_img:
                    sign_image(img + 1)
                pd3 = padded[img]

                for nt in range(N_ROW_TILES):
                    y0 = nt * ROWS_PER_TILE
                    cv = cpsum_pool.tile([O, N_TILE], F32, tag="cv")
                    if USE_FP8:
                        # vertical tap pairs (ky=0,1) x 3 kx
                        for kx in range(KS):
                            win0 = pd3[:, y0 : y0 + ROWS_PER_TILE, kx : kx + W]
                            nc.tensor.matmul(
                                cv,
                                bw_pair[:, kx],
                                pair_ap(win0, WP),
                                start=(kx == 0),
                                stop=False,
                                perf_mode=mybir.MatmulPerfMode.DoubleRow,
                            )
                        if USE_HPAIR:
                            # horizontal pair: taps (2,0)+(2,1), 1B pair step
                            winh = pd3[
                                :, y0 + 2 : y0 + 2 + ROWS_PER_TILE, 0:W
                            ]
                            nc.tensor.matmul(
                                cv,
                                bw_single[:, 0:2, :],
                                pair_ap(winh, 1),
                                start=False,
                                stop=False,
                                perf_mode=mybir.MatmulPerfMode.DoubleRow,
                            )
                            win = pd3[
                                :, y0 + 2 : y0 + 2 + ROWS_PER_TILE, 2 : 2 + W
                            ]
                            nc.tensor.matmul(
                                cv, bw_single[:, 2, :], win,
                                start=False, stop=True,
                            )
                        else:
                            for kx in range(KS):
                                win = pd3[
                                    :, y0 + 2 : y0 + 2 + ROWS_PER_TILE,
                                    kx : kx + W,
                                ]
                                nc.tensor.matmul(
                                    cv, bw_single[:, kx, :], win,
                                    start=False, stop=(kx == KS - 1),
                                )
                    else:
                        t = 0
                        for ky in range(KS):
                            for kx in range(KS):
                                win = pd3[
                                    :,
                                    y0 + ky : y0 + ky + ROWS_PER_TILE,
                                    kx : kx + W,
                                ]
                                nc.tensor.matmul(
                                    cv,
                                    bw_lhsT[:, t, :],
                                    win,
                                    start=(t == 0),
                                    stop=(t == NTAPS - 1),
                                )
                                t += 1
                    ev = ev_pool.tile([O, N_TILE], F16, tag="ev")
                    ev3 = ev.rearrange("o (h w) -> o h w", w=W)
                    last_tile = (
                        img == n_img - 1 and nt == N_ROW_TILES - 1
                    )
                    if last_tile:
                        # split the final evac across DVE+ACT (separate ev
                        # tiles: same-tile WAW serializes at whole-tile
                        # granularity) and the final store across both
                        # rings: halves the end-of-kernel serial tail
                        nh = N_TILE // 2
                        hr = ROWS_PER_TILE // 2
                        evb = ev_pool.tile([O, nh], F16, tag="evb", bufs=1)
                        nc.vector.tensor_scalar_mul(
                            ev[:, 0:nh], cv[:, 0:nh], alpha_sb[:, 0:1]
                        )
                        nc.scalar.mul(
                            evb, cv[:, nh:N_TILE], alpha_sb[:, 0:1]
                        )
                        nc.scalar.dma_start(
                            out_t[img, :, y0 : y0 + hr, :], ev3[:, 0:hr, :]
                        )
                        nc.gpsimd.dma_start(
                            out_t[img, :, y0 + hr : y0 + ROWS_PER_TILE, :],
                            evb.rearrange("o (h w) -> o h w", w=W),
                        )
                    else:
                        nc.vector.tensor_scalar_mul(ev, cv, alpha_sb[:, 0:1])
                        # stores on their own queues: never head-of-line
                        # block the x loads riding the sync queue
                        dma_eng = nc.scalar if (nt % 2 == 0) else nc.gpsimd
                        dma_eng.dma_start(
                            out_t[img, :, y0 : y0 + ROWS_PER_TILE, :], ev3
                        )

    nc.compile()
    return nc


def _ensure_ntff_hook():
    """Register the axon NTFF profiling hook if the image's antenv lacks it.

    Only used when BASS_KERNEL_TRACE=1 (dev profiling); best-effort.
    """
    import sys
    import types

    try:
        import antenv

        if hasattr(antenv, "axon_hooks"):
            return
        mod = types.ModuleType("antenv.axon_hooks")
        _hook = [None]
        mod.set_axon_ntff_profile_hook = lambda h: _hook.__setitem__(0, h)
        mod.get_axon_ntff_profile_hook = lambda: _hook[0]
        sys.modules["antenv.axon_hooks"] = mod
        antenv.axon_hooks = mod
        from trn_agent_boot.trn_boot import _ntff_profile_via_ctypes

        mod.set_axon_ntff_profile_hook(
            _ntff_profile_via_ctypes("/opt/axon/libaxon_pjrt.so")
        )
    except Exception as e:  # pragma: no cover - profiling is optional
        print(f"NTFF hook registration failed ({e}); tracing disabled")


def kernel(x, Alpha, M, Z, rv):
    x = np.ascontiguousarray(np.asarray(x, dtype=np.float32))
    Alpha = np.ascontiguousarray(np.asarray(Alpha, dtype=np.float32))
    M = np.ascontiguousarray(np.asarray(M, dtype=np.float32))
    Z = np.ascontiguousarray(np.asarray(Z, dtype=np.float32))
    rv = np.asarray(rv, dtype=np.float32)

    trace = bool(int(os.environ.get("BASS_KERNEL_TRACE", "0")))
    if trace:
        _ensure_ntff_hook()

    nc = build_program(rv)

    in_maps = []
    for c in range(N_CORES):
        if USE_AG:
            m_c = np.ascontiguousarray(M[:, c * C_SH : (c + 1) * C_SH])
            z_c = np.ascontiguousarray(Z[:, :, c * C_SH : (c + 1) * C_SH])
        else:
            m_c, z_c = M, Z
        in_maps.append(
            {
                "x": np.ascontiguousarray(x[c * B_CORE : (c + 1) * B_CORE]),
                "Alpha": Alpha,
                "M": m_c,
                "Z": z_c,
            }
        )

    res = run_bass_kernel_spmd(
        nc,
        in_maps,
        core_ids=list(range(N_CORES)),
        trace=trace,
    )
    out = np.concatenate(
        [res.results[c]["out"] for c in range(N_CORES)], axis=0
    ).astype(np.float32)
    if trace:
        kernel.last_results = res
    return out


# revision 54
# speedup vs baseline: 1.0080x; 1.0080x over previous
"""Trainium2 Bass kernel for BinarizeConv2dSDP.

Math (reference):
    s   = M + rv @ Z          (the rsqrt normalization is sign-preserving:
                               w = (m + rv@z) * rsqrt(...) with rsqrt > 0,
                               so sign(w) == sign(s))
    bw  = sign(s)             (O, I, 3, 3)
    ba  = sign(x)             (B, C, H, W)
    out = conv2d(ba, bw, stride 1, pad 1) * Alpha

Strategy (69.5us baseline -> ~59-61us; HW exec varies +-1us run-to-run and
the device occasionally sits in a ~68us slow regime for identical code):
    - Data-parallel over batch: 8 cores x 4 images each. M/Z/Alpha replicated.
    - Head pipeline: weights-first DMA order on the sync ring
      [M, z0-z4, alpha, x0 strips, x1 strips]; x2/x3 issue from
      the conv loop. Weight chain s = M + sum_k rv_k Z_k runs k-major on the
      DVE over per-ky chunk tiles (whole-tile dep granularity), consuming
      each z_k as it lands; per chunk: ACT sign -> 3 full [128x128] PE
      transposes -> packed-psum copy. x0/x1+ arrive as row strips in
      separate tiles so each strip's sign starts as it lands; pd deps are
      subtile-level, so conv tile t needs only the strips covering its
      rows (x0 signs are emitted AFTER the weight signs -- the static
      scheduler once ordered x0s0's sign before c2's, delaying the pack
      that gates the conv's horizontal-pair/single passes).
    - Binarized conv: sign(x) stored fp8e4 in a zero-padded [128, 58 x 64]
      SBUF image (row stride 64). Per output row-block, 5 PE passes:
      3 vertical-pair DoubleRow matmuls (K=256, pair step 64B), 1
      horizontal-pair DoubleRow for taps (2,0)+(2,1) (pair step 1B!), 1
      single matmul for tap (2,2). +-1 is exact in fp8e4/bf16; PSUM f32.
      Measured ~194ns per N=448 matmul, gapless.
    - Evac applies Alpha and writes float16 (conv values are integers
      <= 1152, exact in f16; only the Alpha scale rounds, ~2^-12); stores
      ride scalar/gpsimd queues. Host casts back to f32. The very last
      tile's evac is split across DVE+ACT into two ev tiles and stored as
      two half-row-blocks on both rings to shorten the end-of-kernel
      serial tail (ACT's activation Copy takes Alpha as a per-partition
      scale vector).
    - Measured dead ends kept off: in-channel-sharded synthesis + HBM
      AllGather (USE_AG) stalls ~77us on the collective; cross-ring DMA
      (x0 or z halves on the scalar ring) costs 1-4us; alpha issued first
      delays the z stream; alpha issued last raced the first evac once.
"""

import os
import numpy as np

import concourse.bass as bass
import concourse.tile as tile
from concourse import bacc, mybir
from concourse.bass_utils import run_bass_kernel_spmd
from concourse.masks import make_identity

F32 = mybir.dt.float32
F16 = mybir.dt.float16
BF16 = mybir.dt.bfloat16
FP8 = mybir.dt.float8e4

USE_FP8 = bool(int(os.environ.get("BASS_KERNEL_FP8", "1")))
# 5-pass conv (horizontal DoubleRow pair with 1-byte pair stride). If HW
# rejects the 1B pair offset, set to 0 for the 6-pass fallback.
USE_HPAIR = bool(int(os.environ.get("BASS_KERNEL_HPAIR", "1")))
# Shard the weight synthesis by in-channel across the 8 cores and AllGather
# the packed fp8 lhsT (18KB/core) via HBM. Measured DISASTER on this
# runtime: the collective stalls the conv ~77us (rendezvous/launch
# overhead), 130us total vs 60us without. Kept for reference, default off.
USE_AG = USE_FP8 and bool(int(os.environ.get("BASS_KERNEL_AG", "0")))

B_FULL = 32
N_CORES = 8
B_CORE = B_FULL // N_CORES  # 4 images per core
C = 128      # in channels
O = 128      # out channels
H = W = 56
HP = 58                      # padded rows
WP = 64 if USE_FP8 else 58   # padded row stride
KS = 3
NTAPS = KS * KS
IKK = C * NTAPS  # 1152
ROWS_PER_TILE = 8           # output rows per PSUM tile -> N = 8*56 = 448
N_TILE = ROWS_PER_TILE * W  # 448 fp32 <= 512 (one PSUM bank)
N_ROW_TILES = H // ROWS_PER_TILE  # 7
ADT = FP8 if USE_FP8 else BF16

# Weight-chain chunks are TAP-major (one kernel row ky per chunk, all 128
# channels) so each chunk's transposes are 3 full [128,128] PE transposes
# instead of 9 narrow ones. Pool (gpsimd) rejects InstTensorScalarPtr on
# TRN2, so the whole chain runs on the DVE: 3 chunk-ops/k (~1.4us) matches
# the ~1.7us per-z DMA cadence.
NCHUNK = KS                  # chunk g covers taps ky==g (384 elems/partition)
# x0 row strips: pd deps are subtile-level, so conv tile t only needs the
# strips covering its rows; tile0 reads x rows 0..8, so a 10-row first
# strip covers it and lands/signs sooner
X0_STRIPS = (10, 16, 16, 14)
C_SH = C // N_CORES          # in-channels synthesized per core under AG


def build_program(rv: np.ndarray, n_img: int = B_CORE):
    """Build the per-core Bass program. rv values are baked as immediates."""
    nc = bacc.Bacc(
        "TRN2",
        target_bir_lowering=False,
        debug=False,
        num_devices=N_CORES,
    )

    x_t = nc.dram_tensor("x", (n_img, C, H, W), F32, kind="ExternalInput").ap()
    a_t = nc.dram_tensor("Alpha", (O, 1, 1), F32, kind="ExternalInput").ap()
    CW = C_SH if USE_AG else C  # channel width this core synthesizes
    m_t = nc.dram_tensor("M", (O, CW, KS, KS), F32, kind="ExternalInput").ap()
    z_t = nc.dram_tensor("Z", (5, O, CW, KS, KS), F32, kind="ExternalInput").ap()
    out_t = nc.dram_tensor("out", (n_img, O, H, W), F16, kind="ExternalOutput").ap()
    if USE_AG:
        bwg_in_t = nc.dram_tensor(
            "bwg_in", (C_SH, IKK), FP8, kind="Internal"
        ).ap()
        bwg_out_t = nc.dram_tensor(
            "bwg_out", (C, IKK), FP8, kind="Internal", addr_space="Shared"
        ).ap()

    rv = np.asarray(rv, dtype=np.float32).reshape(-1)
    assert rv.shape[0] == 5

    x_flat = x_t.rearrange("n c h w -> n c (h w)")

    with tile.TileContext(nc) as tc:
        with (
            tc.tile_pool(name="const", bufs=1) as const_pool,
            tc.tile_pool(name="wsyn", bufs=1) as wsyn_pool,
            tc.tile_pool(name="imgs", bufs=1) as img_pool,
            tc.tile_pool(name="xstage", bufs=1) as x_pool,
            tc.tile_pool(name="evac", bufs=8) as ev_pool,
            tc.tile_pool(name="cpsum", bufs=6, space="PSUM") as cpsum_pool,
            tc.tile_pool(name="tpsum", bufs=1, space="PSUM") as tpsum_pool,
        ):
            # ---- head DMA issue: weights first, x0 strips interleaved ----
            alpha_sb = const_pool.tile([O, 1], F32)
            GIKK = CW * NTAPS  # weight elems per partition this core owns
            m_sb = wsyn_pool.tile([O, GIKK], F32)
            nc.sync.dma_start(m_sb, m_t.rearrange("o i kh kw -> o (i kh kw)"))
            z_sbs = []

            def dma_z(k):
                z_sb = wsyn_pool.tile([O, GIKK], F32, name=f"z{k}", tag=f"z{k}")
                nc.sync.dma_start(
                    z_sb, z_t[k].rearrange("o i kh kw -> o (i kh kw)")
                )
                z_sbs.append(z_sb)

            # x0 comes in separate strip tiles so each strip's sign can start
            # as soon as that strip lands (whole-tile dep granularity).
            x0_strip = [
                x_pool.tile([C, nr * W], F32, name=f"x0s{i}", tag=f"x0s{i}")
                for i, nr in enumerate(X0_STRIPS)
            ]
            x0_r0 = [sum(X0_STRIPS[:i]) for i in range(len(X0_STRIPS))]

            def dma_x0_strip(i, eng=None):
                (eng or nc.sync).dma_start(
                    x0_strip[i],
                    x_flat[0, :, x0_r0[i] * W : (x0_r0[i] + X0_STRIPS[i]) * W],
                )

            if USE_AG:
                for k in range(5):
                    dma_z(k)
                for i in range(len(X0_STRIPS)):
                    dma_x0_strip(i)
                nc.sync.dma_start(
                    alpha_sb, a_t.rearrange("o a b -> o (a b)")
                )
            else:
                for k in range(5):
                    dma_z(k)
                # alpha here: lands ~6us before the first evac reads it
                # (late placement raced the evac once; cross-ring placement
                # costs ~1-3us)
                nc.sync.dma_start(
                    alpha_sb, a_t.rearrange("o a b -> o (a b)")
                )
                for i in range(len(X0_STRIPS)):
                    dma_x0_strip(i)
            # images 1..n-1 stream as two strips each so their signs start
            # as soon as each strip lands (pd ready ~strip-sign after land).
            # Only x1 is issued up front: a deep backlog of outstanding DMA
            # instructions slows the PE ~2x (observed), so x2/x3 issue
            # lazily from inside the conv loop.
            XI_STRIPS = (28, 28)
            xi_r0 = (0, 28)
            x_strips = {}

            def dma_image(img):
                for j, nr in enumerate(XI_STRIPS):
                    t = x_pool.tile(
                        [C, nr * W], F32, name=f"x{img}s{j}", tag=f"x{img}s{j}"
                    )
                    nc.sync.dma_start(
                        t, x_flat[img, :, xi_r0[j] * W : (xi_r0[j] + nr) * W]
                    )
                    x_strips[(img, j)] = t

            if n_img > 1:
                dma_image(1)

            def sign_image(img):
                pd3 = padded[img]
                for j, nr in enumerate(XI_STRIPS):
                    r0 = xi_r0[j]
                    nc.scalar.sign(
                        pd3[:, 1 + r0 : 1 + r0 + nr, 1 : 1 + W],
                        x_strips[(img, j)].rearrange("c (h w) -> c h w", w=W),
                    )

            identity = const_pool.tile([128, 128], BF16)
            make_identity(nc, identity)


            # ---- per-image padded sign(x) buffers (borders zeroed once) ----
            padded = []
            for img in range(n_img):
                pd = img_pool.tile(
                    [C, HP * WP], ADT, name=f"pad{img}", tag=f"pad{img}"
                )
                pd3 = pd.rearrange("p (h w) -> p h w", w=WP)
                nc.gpsimd.memset(pd3[:, 0, 0:HP], 0.0)
                nc.gpsimd.memset(pd3[:, HP - 1, 0:HP], 0.0)
                nc.gpsimd.memset(pd3[:, 1 : HP - 1, 0:1], 0.0)
                nc.gpsimd.memset(pd3[:, 1 : HP - 1, HP - 1 : HP], 0.0)
                padded.append(pd3)

            def sign_x0_strip(i):
                r0 = x0_r0[i]
                nc.scalar.sign(
                    padded[0][:, 1 + r0 : 1 + r0 + X0_STRIPS[i], 1 : 1 + W],
                    x0_strip[i].rearrange("c (h w) -> c h w", w=W),
                )

            if USE_AG:
                # ---- sharded weight synthesis: this core synthesizes its
                # C_SH in-channel slice, packs it as the final fp8 lhsT row
                # block, AllGathers via HBM, and loads the full lhsT back ----
                s_sb = wsyn_pool.tile([O, GIKK], F32)
                for k in range(5):
                    nc.vector.scalar_tensor_tensor(
                        out=s_sb,
                        in0=z_sbs[k],
                        scalar=float(rv[k]),
                        in1=m_sb if k == 0 else s_sb,
                        op0=mybir.AluOpType.mult,
                        op1=mybir.AluOpType.add,
                    )
                bwn = wsyn_pool.tile([O, GIKK], BF16)
                nc.scalar.sign(bwn, s_sb)
                bwn3 = bwn.rearrange("o (i t) -> o i t", t=NTAPS)
                tpP = tpsum_pool.tile([C_SH, KS * 2 * O], BF16)
                tpS = tpsum_pool.tile([C_SH, KS * O], BF16)
                tpP4 = tpP.rearrange("p (a b o) -> p a b o", b=2, o=O)
                tpS3 = tpS.rearrange("p (a o) -> p a o", o=O)
                for t in range(NTAPS):
                    ky, kx = divmod(t, KS)
                    dst = tpS3[:, kx, :] if ky == 2 else tpP4[:, kx, ky, :]
                    nc.tensor.transpose(dst, bwn3[:, :, t], identity)
                bw_my = wsyn_pool.tile([C_SH, IKK], FP8)
                nc.scalar.copy(bw_my[:, 0 : KS * 2 * O], tpP)
                nc.vector.tensor_copy(bw_my[:, KS * 2 * O : IKK], tpS)
                nc.sync.dma_start(bwg_in_t, bw_my)
                nc.gpsimd.collective_compute(
                    "AllGather",
                    mybir.AluOpType.bypass,
                    replica_groups=[list(range(N_CORES))],
                    ins=[bwg_in_t],
                    outs=[bwg_out_t],
                )
                bw_all = wsyn_pool.tile([C, IKK], FP8)
                nc.sync.dma_start(bw_all, bwg_out_t)
                bw_pair = bw_all[:, 0 : KS * 2 * O].rearrange(
                    "p (a b o) -> p a b o", b=2, o=O
                )
                bw_single = bw_all[:, KS * 2 * O : IKK].rearrange(
                    "p (a o) -> p a o", o=O
                )
                for i in range(len(X0_STRIPS)):
                    sign_x0_strip(i)
            else:
                # ---- full weight synthesis: s = M + sum_k rv_k Z_k, k-major
                # over per-chunk (per-ky) tiles so each z_k is consumed as it
                # lands ----
                GSZ = C * KS  # 384 elems per partition per chunk
                m3 = m_sb.rearrange("o (i t) -> o i t", t=NTAPS)
                z3s = [
                    z.rearrange("o (i t) -> o i t", t=NTAPS) for z in z_sbs
                ]
                s_c = [
                    wsyn_pool.tile([O, GSZ], F32, name=f"s{g}", tag=f"s{g}")
                    for g in range(NCHUNK)
                ]
                bw_c = [
                    wsyn_pool.tile([O, GSZ], BF16, name=f"bw{g}", tag=f"bw{g}")
                    for g in range(NCHUNK)
                ]
                for k in range(5):
                    for g in range(NCHUNK):
                        tsl = slice(g * KS, (g + 1) * KS)
                        nc.vector.scalar_tensor_tensor(
                            out=s_c[g].rearrange("o (i t) -> o i t", t=KS),
                            in0=z3s[k][:, :, tsl],
                            scalar=float(rv[k]),
                            in1=m3[:, :, tsl]
                            if k == 0
                            else s_c[g].rearrange("o (i t) -> o i t", t=KS),
                            op0=mybir.AluOpType.mult,
                            op1=mybir.AluOpType.add,
                        )

                # per-chunk: sign -> 3 full-width PE transposes -> pack copy.
                # fp8 psum layout: tpP[(kx, ky<2, o)] pairs, tpS[(kx, o)] the
                # ky=2 taps. bf16 layout: same split (6 + 3 taps).
                if USE_FP8:
                    bw_pair = wsyn_pool.tile([C, KS, 2, O], FP8)
                    bw_single = wsyn_pool.tile([C, KS, O], FP8)
                else:
                    bw_lhsT = wsyn_pool.tile([C, NTAPS, O], BF16)
                tpP = tpsum_pool.tile([128, KS * 2 * O], BF16)
                tpS = tpsum_pool.tile([128, KS * O], BF16)
                tpP4 = tpP.rearrange("p (a b o) -> p a b o", b=2, o=O)
                tpS3 = tpS.rearrange("p (a o) -> p a o", o=O)

                def emit_chunk(g):
                    ky = g
                    nc.scalar.sign(bw_c[g], s_c[g])
                    bw3 = bw_c[g].rearrange("o (i t) -> o i t", t=KS)
                    for kx in range(KS):
                        dst = tpS3[:, kx, :] if ky == 2 else tpP4[:, kx, ky, :]
                        nc.tensor.transpose(dst, bw3[:, :, kx], identity)

                def pack_chunk(g):
                    ky = g
                    if USE_FP8:
                        dst = (
                            bw_single.rearrange("p a o -> p (a o)")
                            if ky == 2
                            else bw_pair[:, :, ky, :]
                        )
                    else:
                        dst = bw_lhsT.rearrange("p (a t) o -> p a t o", a=KS)[
                            :, ky, :, :
                        ]
                    src = tpS if ky == 2 else tpP4[:, :, ky, :]
                    nc.vector.tensor_copy(dst, src)

                emit_chunk(0)
                emit_chunk(1)
                if USE_FP8:
                    # bw_pair's (kx, ky, o) layout is element-identical to
                    # tpP: one contiguous copy instead of two strided ones
                    nc.vector.tensor_copy(
                        bw_pair.rearrange("p a b o -> p (a b o)"), tpP
                    )
                else:
                    pack_chunk(0)
                emit_chunk(2)
                if USE_FP8:
                    nc.vector.tensor_copy(
                        bw_single.rearrange("p a o -> p (a o)"), tpS
                    )
                else:
                    pack_chunk(1)
                    pack_chunk(2)

                # x0 signs emitted after the weight path: the static
                # scheduler once ordered x0s0's sign before c2's, delaying
                # the pack that gates the conv's last two passes
                for i in range(len(X0_STRIPS)):
                    sign_x0_strip(i)

            # ---- main conv loop; next image's sign emitted before this
            # image's tiles so ACT never head-of-line blocks the sign ----
            def pair_ap(win, pair_stride):
                return bass.AP(
                    win.tensor,
                    win.offset,
                    [list(win.ap[0]), [pair_stride, 2]]
                    + [list(p) for p in win.ap[1:]],
                )

            for img in range(n_img):
                if img + 2 < n_img:
                    dma_image(img + 2)
                if img + 1 < n_img:
                    sign_image(img + 1)
                pd3 = padded[img]

                for nt in range(N_ROW_TILES):
                    y0 = nt * ROWS_PER_TILE
                    cv = cpsum_pool.tile([O, N_TILE], F32, tag="cv")
                    if USE_FP8:
                        # vertical tap pairs (ky=0,1) x 3 kx
                        for kx in range(KS):
                            win0 = pd3[:, y0 : y0 + ROWS_PER_TILE, kx : kx + W]
                            nc.tensor.matmul(
                                cv,
                                bw_pair[:, kx],
                                pair_ap(win0, WP),
                                start=(kx == 0),
                                stop=False,
                                perf_mode=mybir.MatmulPerfMode.DoubleRow,
                            )
                        if USE_HPAIR:
                            # horizontal pair: taps (2,0)+(2,1), 1B pair step
                            winh = pd3[
                                :, y0 + 2 : y0 + 2 + ROWS_PER_TILE, 0:W
                            ]
                            nc.tensor.matmul(
                                cv,
                                bw_single[:, 0:2, :],
                                pair_ap(winh, 1),
                                start=False,
                                stop=False,
                                perf_mode=mybir.MatmulPerfMode.DoubleRow,
                            )
                            win = pd3[
                                :, y0 + 2 : y0 + 2 + ROWS_PER_TILE, 2 : 2 + W
                            ]
                            nc.tensor.matmul(
                                cv, bw_single[:, 2, :], win,
                                start=False, stop=True,
                            )
                        else:
                            for kx in range(KS):
                                win = pd3[
                                    :, y0 + 2 : y0 + 2 + ROWS_PER_TILE,
                                    kx : kx + W,
                                ]
                                nc.tensor.matmul(
                                    cv, bw_single[:, kx, :], win,
                                    start=False, stop=(kx == KS - 1),
                                )
                    else:
                        t = 0
                        for ky in range(KS):
                            for kx in range(KS):
                                win = pd3[
                                    :,
                                    y0 + ky : y0 + ky + ROWS_PER_TILE,
                                    kx : kx + W,
                                ]
                                nc.tensor.matmul(
                                    cv,
                                    bw_lhsT[:, t, :],
                                    win,
                                    start=(t == 0),
                                    stop=(t == NTAPS - 1),
                                )
                                t += 1
                    ev = ev_pool.tile([O, N_TILE], F16, tag="ev")
                    ev3 = ev.rearrange("o (h w) -> o h w", w=W)
                    last_tile = (
                        img == n_img - 1 and nt == N_ROW_TILES - 1
                    )
                    if last_tile:
                        # split the final evac across DVE+ACT (separate ev
                        # tiles: same-tile WAW serializes at whole-tile
                        # granularity) and the final store across both
                        # rings: halves the end-of-kernel serial tail
                        nh = N_TILE // 2
                        hr = ROWS_PER_TILE // 2
                        evb = ev_pool.tile([O, nh], F16, tag="evb", bufs=1)
                        nc.vector.tensor_scalar_mul(
                            ev[:, 0:nh], cv[:, 0:nh], alpha_sb[:, 0:1]
                        )
                        nc.scalar.mul(
                            evb, cv[:, nh:N_TILE], alpha_sb[:, 0:1]
                        )
                        nc.scalar.dma_start(
                            out_t[img, :, y0 : y0 + hr, :], ev3[:, 0:hr, :]
                        )
                        nc.gpsimd.dma_start(
                            out_t[img, :, y0 + hr : y0 + ROWS_PER_TILE, :],
                            evb.rearrange("o (h w) -> o h w", w=W),
                        )
                    else:
                        nc.vector.tensor_scalar_mul(ev, cv, alpha_sb[:, 0:1])
                        # stores on their own queues: never head-of-line
                        # block the x loads riding the sync queue
                        dma_eng = nc.scalar if (nt % 2 == 0) else nc.gpsimd
                        dma_eng.dma_start(
                            out_t[img, :, y0 : y0 + ROWS_PER_TILE, :], ev3
                        )

    nc.compile()
    return nc


def _ensure_ntff_hook():
    """Register the axon NTFF profiling hook if the image's antenv lacks it.

    Only used when BASS_KERNEL_TRACE=1 (dev profiling); best-effort.
    """
    import sys
    import types

    try:
        import antenv

        if hasattr(antenv, "axon_hooks"):
            return
        mod = types.ModuleType("antenv.axon_hooks")
        _hook = [None]
        mod.set_axon_ntff_profile_hook = lambda h: _hook.__setitem__(0, h)
        mod.get_axon_ntff_profile_hook = lambda: _hook[0]
        sys.modules["antenv.axon_hooks"] = mod
        antenv.axon_hooks = mod
        from trn_agent_boot.trn_boot import _ntff_profile_via_ctypes

        mod.set_axon_ntff_profile_hook(
            _ntff_profile_via_ctypes("/opt/axon/libaxon_pjrt.so")
        )
    except Exception as e:  # pragma: no cover - profiling is optional
        print(f"NTFF hook registration failed ({e}); tracing disabled")


def kernel(x, Alpha, M, Z, rv):
    x = np.ascontiguousarray(np.asarray(x, dtype=np.float32))
    Alpha = np.ascontiguousarray(np.asarray(Alpha, dtype=np.float32))
    M = np.ascontiguousarray(np.asarray(M, dtype=np.float32))
    Z = np.ascontiguousarray(np.asarray(Z, dtype=np.float32))
    rv = np.asarray(rv, dtype=np.float32)

    trace = bool(int(os.environ.get("BASS_KERNEL_TRACE", "0")))
    if trace:
        _ensure_ntff_hook()

    nc = build_program(rv)

    in_maps = []
    for c in range(N_CORES):
        if USE_AG:
            m_c = np.ascontiguousarray(M[:, c * C_SH : (c + 1) * C_SH])
            z_c = np.ascontiguousarray(Z[:, :, c * C_SH : (c + 1) * C_SH])
        else:
            m_c, z_c = M, Z
        in_maps.append(
            {
                "x": np.ascontiguousarray(x[c * B_CORE : (c + 1) * B_CORE]),
                "Alpha": Alpha,
                "M": m_c,
                "Z": z_c,
            }
        )

    res = run_bass_kernel_spmd(
        nc,
        in_maps,
        core_ids=list(range(N_CORES)),
        trace=trace,
    )
    out = np.concatenate(
        [res.results[c]["out"] for c in range(N_CORES)], axis=0
    ).astype(np.float32)
    if trace:
        kernel.last_results = res
    return out


# revision 55
# speedup vs baseline: 1.0161x; 1.0080x over previous
"""Trainium2 Bass kernel for BinarizeConv2dSDP.

Math (reference):
    s   = M + rv @ Z          (the rsqrt normalization is sign-preserving:
                               w = (m + rv@z) * rsqrt(...) with rsqrt > 0,
                               so sign(w) == sign(s))
    bw  = sign(s)             (O, I, 3, 3)
    ba  = sign(x)             (B, C, H, W)
    out = conv2d(ba, bw, stride 1, pad 1) * Alpha

Strategy (69.5us baseline -> ~59-61us; HW exec varies +-1us run-to-run and
the device occasionally sits in a ~68us slow regime for identical code):
    - Data-parallel over batch: 8 cores x 4 images each. M/Z/Alpha replicated.
    - Head pipeline: weights-first DMA order on the sync ring
      [M, z0-z4, alpha, x0 strips, x1 strips]; x2/x3 issue from
      the conv loop. Weight chain s = M + sum_k rv_k Z_k runs k-major on the
      DVE over per-ky chunk tiles (whole-tile dep granularity), consuming
      each z_k as it lands; per chunk: ACT sign -> 3 full [128x128] PE
      transposes -> packed-psum copy. x0/x1+ arrive as row strips in
      separate tiles so each strip's sign starts as it lands; pd deps are
      subtile-level, so conv tile t needs only the strips covering its
      rows (x0 signs are emitted AFTER the weight signs -- the static
      scheduler once ordered x0s0's sign before c2's, delaying the pack
      that gates the conv's horizontal-pair/single passes).
    - Binarized conv: sign(x) stored fp8e4 in a zero-padded [128, 58 x 64]
      SBUF image (row stride 64). Per output row-block, 5 PE passes:
      3 vertical-pair DoubleRow matmuls (K=256, pair step 64B), 1
      horizontal-pair DoubleRow for taps (2,0)+(2,1) (pair step 1B!), 1
      single matmul for tap (2,2). +-1 is exact in fp8e4/bf16; PSUM f32.
      Measured ~194ns per N=448 matmul, gapless.
    - Evac applies Alpha and writes float16 (conv values are integers
      <= 1152, exact in f16; only the Alpha scale rounds, ~2^-12); stores
      ride scalar/gpsimd queues. Host casts back to f32. The very last
      tile's evac is split across DVE+ACT into two ev tiles and stored as
      two half-row-blocks on both rings to shorten the end-of-kernel
      serial tail (ACT's activation Copy takes Alpha as a per-partition
      scale vector).
    - Measured dead ends kept off: in-channel-sharded synthesis + HBM
      AllGather (USE_AG) stalls ~77us on the collective; cross-ring DMA
      (x0 or z halves on the scalar ring) costs 1-4us; alpha issued first
      delays the z stream; alpha issued last raced the first evac once.
"""

import os
import numpy as np

import concourse.bass as bass
import concourse.tile as tile
from concourse import bacc, mybir
from concourse.bass_utils import run_bass_kernel_spmd
from concourse.masks import make_identity

F32 = mybir.dt.float32
F16 = mybir.dt.float16
BF16 = mybir.dt.bfloat16
FP8 = mybir.dt.float8e4

USE_FP8 = bool(int(os.environ.get("BASS_KERNEL_FP8", "1")))
# 5-pass conv (horizontal DoubleRow pair with 1-byte pair stride). If HW
# rejects the 1B pair offset, set to 0 for the 6-pass fallback.
USE_HPAIR = bool(int(os.environ.get("BASS_KERNEL_HPAIR", "1")))
# Shard the weight synthesis by in-channel across the 8 cores and AllGather
# the packed fp8 lhsT (18KB/core) via HBM. Measured DISASTER on this
# runtime: the collective stalls the conv ~77us (rendezvous/launch
# overhead), 130us total vs 60us without. Kept for reference, default off.
USE_AG = USE_FP8 and bool(int(os.environ.get("BASS_KERNEL_AG", "0")))

B_FULL = 32
N_CORES = 8
B_CORE = B_FULL // N_CORES  # 4 images per core
C = 128      # in channels
O = 128      # out channels
H = W = 56
HP = 58                      # padded rows
WP = 64 if USE_FP8 else 58   # padded row stride
KS = 3
NTAPS = KS * KS
IKK = C * NTAPS  # 1152
ROWS_PER_TILE = 8           # output rows per PSUM tile -> N = 8*56 = 448
N_TILE = ROWS_PER_TILE * W  # 448 fp32 <= 512 (one PSUM bank)
N_ROW_TILES = H // ROWS_PER_TILE  # 7
ADT = FP8 if USE_FP8 else BF16

# Weight-chain chunks are TAP-major (one kernel row ky per chunk, all 128
# channels) so each chunk's transposes are 3 full [128,128] PE transposes
# instead of 9 narrow ones. Pool (gpsimd) rejects InstTensorScalarPtr on
# TRN2, so the whole chain runs on the DVE: 3 chunk-ops/k (~1.4us) matches
# the ~1.7us per-z DMA cadence.
NCHUNK = KS                  # chunk g covers taps ky==g (384 elems/partition)
# x0 row strips: pd deps are subtile-level, so conv tile t only needs the
# strips covering its rows; tile0 reads x rows 0..8, so a 10-row first
# strip covers it and lands/signs sooner
X0_STRIPS = (10, 16, 16, 14)
C_SH = C // N_CORES          # in-channels synthesized per core under AG


def build_program(rv: np.ndarray, n_img: int = B_CORE):
    """Build the per-core Bass program. rv values are baked as immediates."""
    nc = bacc.Bacc(
        "TRN2",
        target_bir_lowering=False,
        debug=False,
        num_devices=N_CORES,
    )

    x_t = nc.dram_tensor("x", (n_img, C, H, W), F32, kind="ExternalInput").ap()
    a_t = nc.dram_tensor("Alpha", (O, 1, 1), F32, kind="ExternalInput").ap()
    CW = C_SH if USE_AG else C  # channel width this core synthesizes
    m_t = nc.dram_tensor("M", (O, CW, KS, KS), F32, kind="ExternalInput").ap()
    z_t = nc.dram_tensor("Z", (5, O, CW, KS, KS), F32, kind="ExternalInput").ap()
    out_t = nc.dram_tensor("out", (n_img, O, H, W), F16, kind="ExternalOutput").ap()
    if USE_AG:
        bwg_in_t = nc.dram_tensor(
            "bwg_in", (C_SH, IKK), FP8, kind="Internal"
        ).ap()
        bwg_out_t = nc.dram_tensor(
            "bwg_out", (C, IKK), FP8, kind="Internal", addr_space="Shared"
        ).ap()

    rv = np.asarray(rv, dtype=np.float32).reshape(-1)
    assert rv.shape[0] == 5

    x_flat = x_t.rearrange("n c h w -> n c (h w)")

    with tile.TileContext(nc) as tc:
        with (
            tc.tile_pool(name="const", bufs=1) as const_pool,
            tc.tile_pool(name="wsyn", bufs=1) as wsyn_pool,
            tc.tile_pool(name="imgs", bufs=1) as img_pool,
            tc.tile_pool(name="xstage", bufs=1) as x_pool,
            tc.tile_pool(name="evac", bufs=8) as ev_pool,
            tc.tile_pool(name="cpsum", bufs=6, space="PSUM") as cpsum_pool,
            tc.tile_pool(name="tpsum", bufs=1, space="PSUM") as tpsum_pool,
        ):
            # ---- head DMA issue: weights first, x0 strips interleaved ----
            alpha_sb = const_pool.tile([O, 1], F32)
            GIKK = CW * NTAPS  # weight elems per partition this core owns
            m_sb = wsyn_pool.tile([O, GIKK], F32)
            nc.sync.dma_start(m_sb, m_t.rearrange("o i kh kw -> o (i kh kw)"))
            z_sbs = []

            def dma_z(k):
                z_sb = wsyn_pool.tile([O, GIKK], F32, name=f"z{k}", tag=f"z{k}")
                nc.sync.dma_start(
                    z_sb, z_t[k].rearrange("o i kh kw -> o (i kh kw)")
                )
                z_sbs.append(z_sb)

            # x0 comes in separate strip tiles so each strip's sign can start
            # as soon as that strip lands (whole-tile dep granularity).
            x0_strip = [
                x_pool.tile([C, nr * W], F32, name=f"x0s{i}", tag=f"x0s{i}")
                for i, nr in enumerate(X0_STRIPS)
            ]
            x0_r0 = [sum(X0_STRIPS[:i]) for i in range(len(X0_STRIPS))]

            def dma_x0_strip(i, eng=None):
                (eng or nc.sync).dma_start(
                    x0_strip[i],
                    x_flat[0, :, x0_r0[i] * W : (x0_r0[i] + X0_STRIPS[i]) * W],
                )

            if USE_AG:
                for k in range(5):
                    dma_z(k)
                for i in range(len(X0_STRIPS)):
                    dma_x0_strip(i)
                nc.sync.dma_start(
                    alpha_sb, a_t.rearrange("o a b -> o (a b)")
                )
            else:
                for k in range(5):
                    dma_z(k)
                # alpha here: lands ~6us before the first evac reads it
                # (late placement raced the evac once; cross-ring placement
                # costs ~1-3us)
                nc.sync.dma_start(
                    alpha_sb, a_t.rearrange("o a b -> o (a b)")
                )
                for i in range(len(X0_STRIPS)):
                    dma_x0_strip(i)
            # images 1..n-1 stream as two strips each so their signs start
            # as soon as each strip lands (pd ready ~strip-sign after land).
            # Only x1 is issued up front: a deep backlog of outstanding DMA
            # instructions slows the PE ~2x (observed), so x2/x3 issue
            # lazily from inside the conv loop.
            XI_STRIPS = (28, 28)
            xi_r0 = (0, 28)
            x_strips = {}

            def dma_image(img):
                for j, nr in enumerate(XI_STRIPS):
                    t = x_pool.tile(
                        [C, nr * W], F32, name=f"x{img}s{j}", tag=f"x{img}s{j}"
                    )
                    nc.sync.dma_start(
                        t, x_flat[img, :, xi_r0[j] * W : (xi_r0[j] + nr) * W]
                    )
                    x_strips[(img, j)] = t

            if n_img > 1:
                dma_image(1)

            def sign_image(img):
                pd3 = padded[img]
                for j, nr in enumerate(XI_STRIPS):
                    r0 = xi_r0[j]
                    nc.scalar.sign(
                        pd3[:, 1 + r0 : 1 + r0 + nr, 1 : 1 + W],
                        x_strips[(img, j)].rearrange("c (h w) -> c h w", w=W),
                    )

            identity = const_pool.tile([128, 128], BF16)
            make_identity(nc, identity)


            # ---- per-image padded sign(x) buffers (borders zeroed once) ----
            padded = []
            for img in range(n_img):
                pd = img_pool.tile(
                    [C, HP * WP], ADT, name=f"pad{img}", tag=f"pad{img}"
                )
                pd3 = pd.rearrange("p (h w) -> p h w", w=WP)
                nc.gpsimd.memset(pd3[:, 0, 0:HP], 0.0)
                nc.gpsimd.memset(pd3[:, HP - 1, 0:HP], 0.0)
                nc.gpsimd.memset(pd3[:, 1 : HP - 1, 0:1], 0.0)
                nc.gpsimd.memset(pd3[:, 1 : HP - 1, HP - 1 : HP], 0.0)
                padded.append(pd3)

            def sign_x0_strip(i):
                r0 = x0_r0[i]
                nc.scalar.sign(
                    padded[0][:, 1 + r0 : 1 + r0 + X0_STRIPS[i], 1 : 1 + W],
                    x0_strip[i].rearrange("c (h w) -> c h w", w=W),
                )

            if USE_AG:
                # ---- sharded weight synthesis: this core synthesizes its
                # C_SH in-channel slice, packs it as the final fp8 lhsT row
                # block, AllGathers via HBM, and loads the full lhsT back ----
                s_sb = wsyn_pool.tile([O, GIKK], F32)
                for k in range(5):
                    nc.vector.scalar_tensor_tensor(
                        out=s_sb,
                        in0=z_sbs[k],
                        scalar=float(rv[k]),
                        in1=m_sb if k == 0 else s_sb,
                        op0=mybir.AluOpType.mult,
                        op1=mybir.AluOpType.add,
                    )
                bwn = wsyn_pool.tile([O, GIKK], BF16)
                nc.scalar.sign(bwn, s_sb)
                bwn3 = bwn.rearrange("o (i t) -> o i t", t=NTAPS)
                tpP = tpsum_pool.tile([C_SH, KS * 2 * O], BF16)
                tpS = tpsum_pool.tile([C_SH, KS * O], BF16)
                tpP4 = tpP.rearrange("p (a b o) -> p a b o", b=2, o=O)
                tpS3 = tpS.rearrange("p (a o) -> p a o", o=O)
                for t in range(NTAPS):
                    ky, kx = divmod(t, KS)
                    dst = tpS3[:, kx, :] if ky == 2 else tpP4[:, kx, ky, :]
                    nc.tensor.transpose(dst, bwn3[:, :, t], identity)
                bw_my = wsyn_pool.tile([C_SH, IKK], FP8)
                nc.scalar.copy(bw_my[:, 0 : KS * 2 * O], tpP)
                nc.vector.tensor_copy(bw_my[:, KS * 2 * O : IKK], tpS)
                nc.sync.dma_start(bwg_in_t, bw_my)
                nc.gpsimd.collective_compute(
                    "AllGather",
                    mybir.AluOpType.bypass,
                    replica_groups=[list(range(N_CORES))],
                    ins=[bwg_in_t],
                    outs=[bwg_out_t],
                )
                bw_all = wsyn_pool.tile([C, IKK], FP8)
                nc.sync.dma_start(bw_all, bwg_out_t)
                bw_pair = bw_all[:, 0 : KS * 2 * O].rearrange(
                    "p (a b o) -> p a b o", b=2, o=O
                )
                bw_single = bw_all[:, KS * 2 * O : IKK].rearrange(
                    "p (a o) -> p a o", o=O
                )
                for i in range(len(X0_STRIPS)):
                    sign_x0_strip(i)
            else:
                # ---- full weight synthesis: s = M + sum_k rv_k Z_k, k-major
                # over per-chunk (per-ky) tiles so each z_k is consumed as it
                # lands ----
                GSZ = C * KS  # 384 elems per partition per chunk
                m3 = m_sb.rearrange("o (i t) -> o i t", t=NTAPS)
                z3s = [
                    z.rearrange("o (i t) -> o i t", t=NTAPS) for z in z_sbs
                ]
                s_c = [
                    wsyn_pool.tile([O, GSZ], F32, name=f"s{g}", tag=f"s{g}")
                    for g in range(NCHUNK)
                ]
                bw_c = [
                    wsyn_pool.tile([O, GSZ], BF16, name=f"bw{g}", tag=f"bw{g}")
                    for g in range(NCHUNK)
                ]
                for k in range(5):
                    for g in range(NCHUNK):
                        tsl = slice(g * KS, (g + 1) * KS)
                        nc.vector.scalar_tensor_tensor(
                            out=s_c[g].rearrange("o (i t) -> o i t", t=KS),
                            in0=z3s[k][:, :, tsl],
                            scalar=float(rv[k]),
                            in1=m3[:, :, tsl]
                            if k == 0
                            else s_c[g].rearrange("o (i t) -> o i t", t=KS),
                            op0=mybir.AluOpType.mult,
                            op1=mybir.AluOpType.add,
                        )

                # per-chunk: sign -> 3 full-width PE transposes -> pack copy.
                # fp8 psum layout: tpP[(kx, ky<2, o)] pairs, tpS[(kx, o)] the
                # ky=2 taps. bf16 layout: same split (6 + 3 taps).
                if USE_FP8:
                    bw_pair = wsyn_pool.tile([C, KS, 2, O], FP8)
                    bw_single = wsyn_pool.tile([C, KS, O], FP8)
                else:
                    bw_lhsT = wsyn_pool.tile([C, NTAPS, O], BF16)
                tpP = tpsum_pool.tile([128, KS * 2 * O], BF16)
                tpS = tpsum_pool.tile([128, KS * O], BF16)
                tpP4 = tpP.rearrange("p (a b o) -> p a b o", b=2, o=O)
                tpS3 = tpS.rearrange("p (a o) -> p a o", o=O)

                def emit_chunk(g):
                    ky = g
                    nc.scalar.sign(bw_c[g], s_c[g])
                    bw3 = bw_c[g].rearrange("o (i t) -> o i t", t=KS)
                    for kx in range(KS):
                        dst = tpS3[:, kx, :] if ky == 2 else tpP4[:, kx, ky, :]
                        nc.tensor.transpose(dst, bw3[:, :, kx], identity)

                def pack_chunk(g):
                    ky = g
                    if USE_FP8:
                        dst = (
                            bw_single.rearrange("p a o -> p (a o)")
                            if ky == 2
                            else bw_pair[:, :, ky, :]
                        )
                    else:
                        dst = bw_lhsT.rearrange("p (a t) o -> p a t o", a=KS)[
                            :, ky, :, :
                        ]
                    src = tpS if ky == 2 else tpP4[:, :, ky, :]
                    nc.vector.tensor_copy(dst, src)

                emit_chunk(0)
                emit_chunk(1)
                pack_chunk(0)
                emit_chunk(2)
                pack_chunk(1)
                pack_chunk(2)

                # x0 signs emitted after the weight path: the static
                # scheduler once ordered x0s0's sign before c2's, delaying
                # the pack that gates the conv's last two passes
                for i in range(len(X0_STRIPS)):
                    sign_x0_strip(i)

            # ---- main conv loop; next image's sign emitted before this
            # image's tiles so ACT never head-of-line blocks the sign ----
            def pair_ap(win, pair_stride):
                return bass.AP(
                    win.tensor,
                    win.offset,
                    [list(win.ap[0]), [pair_stride, 2]]
                    + [list(p) for p in win.ap[1:]],
                )

            for img in range(n_img):
                if img + 2 < n_img:
                    dma_image(img + 2)
                if img + 1 < n_img:
                    sign_image(img + 1)
                pd3 = padded[img]

                for nt in range(N_ROW_TILES):
                    y0 = nt * ROWS_PER_TILE
                    cv = cpsum_pool.tile([O, N_TILE], F32, tag="cv")
                    if USE_FP8:
                        # vertical tap pairs (ky=0,1) x 3 kx
                        for kx in range(KS):
                            win0 = pd3[:, y0 : y0 + ROWS_PER_TILE, kx : kx + W]
                            nc.tensor.matmul(
                                cv,
                                bw_pair[:, kx],
                                pair_ap(win0, WP),
                                start=(kx == 0),
                                stop=False,
                                perf_mode=mybir.MatmulPerfMode.DoubleRow,
                            )
                        if USE_HPAIR:
                            # horizontal pair: taps (2,0)+(2,1), 1B pair step
                            winh = pd3[
                                :, y0 + 2 : y0 + 2 + ROWS_PER_TILE, 0:W
                            ]
                            nc.tensor.matmul(
                                cv,
                                bw_single[:, 0:2, :],
                                pair_ap(winh, 1),
                                start=False,
                                stop=False,
                                perf_mode=mybir.MatmulPerfMode.DoubleRow,
                            )
                            win = pd3[
                                :, y0 + 2 : y0 + 2 + ROWS_PER_TILE, 2 : 2 + W
                            ]
                            nc.tensor.matmul(
                                cv, bw_single[:, 2, :], win,
                                start=False, stop=True,
                            )
                        else:
                            for kx in range(KS):
                                win = pd3[
                                    :, y0 + 2 : y0 + 2 + ROWS_PER_TILE,
                                    kx : kx + W,
                                ]
                                nc.tensor.matmul(
                                    cv, bw_single[:, kx, :], win,
                                    start=False, stop=(kx == KS - 1),
                                )
                    else:
                        t = 0
                        for ky in range(KS):
                            for kx in range(KS):
                                win = pd3[
                                    :,
                                    y0 + ky : y0 + ky + ROWS_PER_TILE,
                                    kx : kx + W,
                                ]
                                nc.tensor.matmul(
                                    cv,
                                    bw_lhsT[:, t, :],
                                    win,
                                    start=(t == 0),
                                    stop=(t == NTAPS - 1),
                                )
                                t += 1
                    ev = ev_pool.tile([O, N_TILE], F16, tag="ev")
                    ev3 = ev.rearrange("o (h w) -> o h w", w=W)
                    last_tile = (
                        img == n_img - 1 and nt == N_ROW_TILES - 1
                    )
                    if last_tile:
                        # split the final evac across DVE+ACT (separate ev
                        # tiles: same-tile WAW serializes at whole-tile
                        # granularity) and the final store across both
                        # rings: halves the end-of-kernel serial tail
                        nh = N_TILE // 2
                        hr = ROWS_PER_TILE // 2
                        evb = ev_pool.tile([O, nh], F16, tag="evb", bufs=1)
                        nc.vector.tensor_scalar_mul(
                            ev[:, 0:nh], cv[:, 0:nh], alpha_sb[:, 0:1]
                        )
                        nc.scalar.mul(
                            evb, cv[:, nh:N_TILE], alpha_sb[:, 0:1]
                        )
                        nc.scalar.dma_start(
                            out_t[img, :, y0 : y0 + hr, :], ev3[:, 0:hr, :]
                        )
                        nc.gpsimd.dma_start(
                            out_t[img, :, y0 + hr : y0 + ROWS_PER_TILE, :],
                            evb.rearrange("o (h w) -> o h w", w=W),
                        )
                    else:
                        nc.vector.tensor_scalar_mul(ev, cv, alpha_sb[:, 0:1])
                        # stores on their own queues: never head-of-line
                        # block the x loads riding the sync queue
                        dma_eng = nc.scalar if (nt % 2 == 0) else nc.gpsimd
                        dma_eng.dma_start(
                            out_t[img, :, y0 : y0 + ROWS_PER_TILE, :], ev3
                        )

    nc.compile()
    return nc


def _ensure_ntff_hook():
    """Register the axon NTFF profiling hook if the image's antenv lacks it.

    Only used when BASS_KERNEL_TRACE=1 (dev profiling); best-effort.
    """
    import sys
    import types

    try:
        import antenv

        if hasattr(antenv, "axon_hooks"):
            return
        mod = types.ModuleType("antenv.axon_hooks")
        _hook = [None]
        mod.set_axon_ntff_profile_hook = lambda h: _hook.__setitem__(0, h)
        mod.get_axon_ntff_profile_hook = lambda: _hook[0]
        sys.modules["antenv.axon_hooks"] = mod
        antenv.axon_hooks = mod
        from trn_agent_boot.trn_boot import _ntff_profile_via_ctypes

        mod.set_axon_ntff_profile_hook(
            _ntff_profile_via_ctypes("/opt/axon/libaxon_pjrt.so")
        )
    except Exception as e:  # pragma: no cover - profiling is optional
        print(f"NTFF hook registration failed ({e}); tracing disabled")


def kernel(x, Alpha, M, Z, rv):
    x = np.ascontiguousarray(np.asarray(x, dtype=np.float32))
    Alpha = np.ascontiguousarray(np.asarray(Alpha, dtype=np.float32))
    M = np.ascontiguousarray(np.asarray(M, dtype=np.float32))
    Z = np.ascontiguousarray(np.asarray(Z, dtype=np.float32))
    rv = np.asarray(rv, dtype=np.float32)

    trace = bool(int(os.environ.get("BASS_KERNEL_TRACE", "0")))
    if trace:
        _ensure_ntff_hook()

    nc = build_program(rv)

    in_maps = []
    for c in range(N_CORES):
        if USE_AG:
            m_c = np.ascontiguousarray(M[:, c * C_SH : (c + 1) * C_SH])
            z_c = np.ascontiguousarray(Z[:, :, c * C_SH : (c + 1) * C_SH])
        else:
            m_c, z_c = M, Z
        in_maps.append(
            {
                "x": np.ascontiguousarray(x[c * B_CORE : (c + 1) * B_CORE]),
                "Alpha": Alpha,
                "M": m_c,
                "Z": z_c,
            }
        )

    res = run_bass_kernel_spmd(
        nc,
        in_maps,
        core_ids=list(range(N_CORES)),
        trace=trace,
    )
    out = np.concatenate(
        [res.results[c]["out"] for c in range(N_CORES)], axis=0
    ).astype(np.float32)
    if trace:
        kernel.last_results = res
    return out


# revision 56
# speedup vs baseline: 1.0525x; 1.0358x over previous
"""Trainium2 Bass kernel for BinarizeConv2dSDP.

Math (reference):
    s   = M + rv @ Z          (the rsqrt normalization is sign-preserving:
                               w = (m + rv@z) * rsqrt(...) with rsqrt > 0,
                               so sign(w) == sign(s))
    bw  = sign(s)             (O, I, 3, 3)
    ba  = sign(x)             (B, C, H, W)
    out = conv2d(ba, bw, stride 1, pad 1) * Alpha

Strategy (69.5us baseline -> ~59-61us; HW exec varies +-1us run-to-run and
the device occasionally sits in a ~68us slow regime for identical code):
    - Data-parallel over batch: 8 cores x 4 images each. M/Z/Alpha replicated.
    - Head pipeline: weights-first DMA order on the sync ring
      [M, z0-z4, alpha, x0 strips, x1 strips]; x2/x3 issue from
      the conv loop. Weight chain s = M + sum_k rv_k Z_k runs k-major on the
      DVE over per-ky chunk tiles (whole-tile dep granularity), consuming
      each z_k as it lands; per chunk: ACT sign -> 3 full [128x128] PE
      transposes -> packed-psum copy. x0/x1+ arrive as row strips in
      separate tiles so each strip's sign starts as it lands; pd deps are
      subtile-level, so conv tile t needs only the strips covering its
      rows (x0 signs are emitted AFTER the weight signs -- the static
      scheduler once ordered x0s0's sign before c2's, delaying the pack
      that gates the conv's horizontal-pair/single passes).
    - Binarized conv: sign(x) stored fp8e4 in a zero-padded [128, 58 x 64]
      SBUF image (row stride 64). Per output row-block, 5 PE passes:
      3 vertical-pair DoubleRow matmuls (K=256, pair step 64B), 1
      horizontal-pair DoubleRow for taps (2,0)+(2,1) (pair step 1B!), 1
      single matmul for tap (2,2). +-1 is exact in fp8e4/bf16; PSUM f32.
      Measured ~194ns per N=448 matmul, gapless.
    - Evac applies Alpha and writes float16 (conv values are integers
      <= 1152, exact in f16; only the Alpha scale rounds, ~2^-12); stores
      ride scalar/gpsimd queues. Host casts back to f32. The very last
      tile's evac is split across DVE+ACT into two ev tiles and stored as
      two half-row-blocks on both rings to shorten the end-of-kernel
      serial tail (ACT's activation Copy takes Alpha as a per-partition
      scale vector).
    - Measured dead ends kept off: in-channel-sharded synthesis + HBM
      AllGather (USE_AG) stalls ~77us on the collective; cross-ring DMA
      (x0 or z halves on the scalar ring) costs 1-4us; alpha issued first
      delays the z stream; alpha issued last raced the first evac once;
      merging the two bw_pair pack copies into one contiguous copy
      trended ~+1us despite being analytically neutral; whole-image
      x2/x3 loads (vs strips) regressed ~6us; PE warm-up dummies for the
      early-body 2x-slow epoch made the whole body throttle; z-DMA
      pairing loses in chain pipelining what it saves in ring turnaround.
"""

import os
import numpy as np

import concourse.bass as bass
import concourse.tile as tile
from concourse import bacc, mybir
from concourse.bass_utils import run_bass_kernel_spmd
from concourse.masks import make_identity

F32 = mybir.dt.float32
F16 = mybir.dt.float16
BF16 = mybir.dt.bfloat16
FP8 = mybir.dt.float8e4

USE_FP8 = bool(int(os.environ.get("BASS_KERNEL_FP8", "1")))
# 5-pass conv (horizontal DoubleRow pair with 1-byte pair stride). If HW
# rejects the 1B pair offset, set to 0 for the 6-pass fallback.
USE_HPAIR = bool(int(os.environ.get("BASS_KERNEL_HPAIR", "1")))
# Shard the weight synthesis by in-channel across the 8 cores and AllGather
# the packed fp8 lhsT (18KB/core) via HBM. Measured DISASTER on this
# runtime: the collective stalls the conv ~77us (rendezvous/launch
# overhead), 130us total vs 60us without. Kept for reference, default off.
USE_AG = USE_FP8 and bool(int(os.environ.get("BASS_KERNEL_AG", "0")))

B_FULL = 32
N_CORES = 8
B_CORE = B_FULL // N_CORES  # 4 images per core
C = 128      # in channels
O = 128      # out channels
H = W = 56
HP = 58                      # padded rows
WP = 64 if USE_FP8 else 58   # padded row stride
KS = 3
NTAPS = KS * KS
IKK = C * NTAPS  # 1152
ROWS_PER_TILE = 8           # output rows per PSUM tile -> N = 8*56 = 448
N_TILE = ROWS_PER_TILE * W  # 448 fp32 <= 512 (one PSUM bank)
N_ROW_TILES = H // ROWS_PER_TILE  # 7
ADT = FP8 if USE_FP8 else BF16

# Weight-chain chunks are TAP-major (one kernel row ky per chunk, all 128
# channels) so each chunk's transposes are 3 full [128,128] PE transposes
# instead of 9 narrow ones. Pool (gpsimd) rejects InstTensorScalarPtr on
# TRN2, so the whole chain runs on the DVE: 3 chunk-ops/k (~1.4us) matches
# the ~1.7us per-z DMA cadence.
NCHUNK = KS                  # chunk g covers taps ky==g (384 elems/partition)
# x0 row strips: pd deps are subtile-level, so conv tile t only needs the
# strips covering its rows; tile0 reads x rows 0..8, so a 10-row first
# strip covers it and lands/signs sooner
X0_STRIPS = (10, 16, 16, 14)
C_SH = C // N_CORES          # in-channels synthesized per core under AG


def build_program(rv: np.ndarray, n_img: int = B_CORE):
    """Build the per-core Bass program. rv values are baked as immediates."""
    nc = bacc.Bacc(
        "TRN2",
        target_bir_lowering=False,
        debug=False,
        num_devices=N_CORES,
    )

    x_t = nc.dram_tensor("x", (n_img, C, H, W), F32, kind="ExternalInput").ap()
    a_t = nc.dram_tensor("Alpha", (O, 1, 1), F32, kind="ExternalInput").ap()
    CW = C_SH if USE_AG else C  # channel width this core synthesizes
    m_t = nc.dram_tensor("M", (O, CW, KS, KS), F32, kind="ExternalInput").ap()
    z_t = nc.dram_tensor("Z", (5, O, CW, KS, KS), F32, kind="ExternalInput").ap()
    out_t = nc.dram_tensor("out", (n_img, O, H, W), F16, kind="ExternalOutput").ap()
    if USE_AG:
        bwg_in_t = nc.dram_tensor(
            "bwg_in", (C_SH, IKK), FP8, kind="Internal"
        ).ap()
        bwg_out_t = nc.dram_tensor(
            "bwg_out", (C, IKK), FP8, kind="Internal", addr_space="Shared"
        ).ap()

    rv = np.asarray(rv, dtype=np.float32).reshape(-1)
    assert rv.shape[0] == 5

    x_flat = x_t.rearrange("n c h w -> n c (h w)")

    with tile.TileContext(nc) as tc:
        with (
            tc.tile_pool(name="const", bufs=1) as const_pool,
            tc.tile_pool(name="wsyn", bufs=1) as wsyn_pool,
            tc.tile_pool(name="imgs", bufs=1) as img_pool,
            tc.tile_pool(name="xstage", bufs=1) as x_pool,
            tc.tile_pool(name="evac", bufs=8) as ev_pool,
            tc.tile_pool(name="cpsum", bufs=6, space="PSUM") as cpsum_pool,
            tc.tile_pool(name="tpsum", bufs=1, space="PSUM") as tpsum_pool,
        ):
            # ---- head DMA issue: weights first, x0 strips interleaved ----
            alpha_sb = const_pool.tile([O, 1], F32)
            GIKK = CW * NTAPS  # weight elems per partition this core owns
            m_sb = wsyn_pool.tile([O, GIKK], F32)
            nc.sync.dma_start(m_sb, m_t.rearrange("o i kh kw -> o (i kh kw)"))
            z_sbs = []

            def dma_z(k):
                z_sb = wsyn_pool.tile([O, GIKK], F32, name=f"z{k}", tag=f"z{k}")
                nc.sync.dma_start(
                    z_sb, z_t[k].rearrange("o i kh kw -> o (i kh kw)")
                )
                z_sbs.append(z_sb)

            # x0 comes in separate strip tiles so each strip's sign can start
            # as soon as that strip lands (whole-tile dep granularity).
            x0_strip = [
                x_pool.tile([C, nr * W], F32, name=f"x0s{i}", tag=f"x0s{i}")
                for i, nr in enumerate(X0_STRIPS)
            ]
            x0_r0 = [sum(X0_STRIPS[:i]) for i in range(len(X0_STRIPS))]

            def dma_x0_strip(i, eng=None):
                (eng or nc.sync).dma_start(
                    x0_strip[i],
                    x_flat[0, :, x0_r0[i] * W : (x0_r0[i] + X0_STRIPS[i]) * W],
                )

            if USE_AG:
                for k in range(5):
                    dma_z(k)
                for i in range(len(X0_STRIPS)):
                    dma_x0_strip(i)
                nc.sync.dma_start(
                    alpha_sb, a_t.rearrange("o a b -> o (a b)")
                )
            else:
                for k in range(5):
                    dma_z(k)
                # alpha here: lands ~6us before the first evac reads it
                # (late placement raced the evac once; cross-ring placement
                # costs ~1-3us)
                nc.sync.dma_start(
                    alpha_sb, a_t.rearrange("o a b -> o (a b)")
                )
                for i in range(len(X0_STRIPS)):
                    dma_x0_strip(i)
            # images 1..n-1 stream as two strips each so their signs start
            # as soon as each strip lands (pd ready ~strip-sign after land).
            # Only x1 is issued up front: a deep backlog of outstanding DMA
            # instructions slows the PE ~2x (observed), so x2/x3 issue
            # lazily from inside the conv loop.
            XI_STRIPS = (28, 28)
            xi_r0 = (0, 28)
            x_strips = {}

            def dma_image(img):
                for j, nr in enumerate(XI_STRIPS):
                    t = x_pool.tile(
                        [C, nr * W], F32, name=f"x{img}s{j}", tag=f"x{img}s{j}"
                    )
                    nc.sync.dma_start(
                        t, x_flat[img, :, xi_r0[j] * W : (xi_r0[j] + nr) * W]
                    )
                    x_strips[(img, j)] = t

            if n_img > 1:
                dma_image(1)

            def sign_image(img):
                pd3 = padded[img]
                for j, nr in enumerate(XI_STRIPS):
                    r0 = xi_r0[j]
                    nc.scalar.sign(
                        pd3[:, 1 + r0 : 1 + r0 + nr, 1 : 1 + W],
                        x_strips[(img, j)].rearrange("c (h w) -> c h w", w=W),
                    )

            identity = const_pool.tile([128, 128], BF16)
            make_identity(nc, identity)


            # ---- per-image padded sign(x) buffers (borders zeroed once) ----
            padded = []
            for img in range(n_img):
                pd = img_pool.tile(
                    [C, HP * WP], ADT, name=f"pad{img}", tag=f"pad{img}"
                )
                pd3 = pd.rearrange("p (h w) -> p h w", w=WP)
                nc.gpsimd.memset(pd3[:, 0, 0:HP], 0.0)
                nc.gpsimd.memset(pd3[:, HP - 1, 0:HP], 0.0)
                nc.gpsimd.memset(pd3[:, 1 : HP - 1, 0:1], 0.0)
                nc.gpsimd.memset(pd3[:, 1 : HP - 1, HP - 1 : HP], 0.0)
                padded.append(pd3)

            def sign_x0_strip(i):
                r0 = x0_r0[i]
                nc.scalar.sign(
                    padded[0][:, 1 + r0 : 1 + r0 + X0_STRIPS[i], 1 : 1 + W],
                    x0_strip[i].rearrange("c (h w) -> c h w", w=W),
                )

            if USE_AG:
                # ---- sharded weight synthesis: this core synthesizes its
                # C_SH in-channel slice, packs it as the final fp8 lhsT row
                # block, AllGathers via HBM, and loads the full lhsT back ----
                s_sb = wsyn_pool.tile([O, GIKK], F32)
                for k in range(5):
                    nc.vector.scalar_tensor_tensor(
                        out=s_sb,
                        in0=z_sbs[k],
                        scalar=float(rv[k]),
                        in1=m_sb if k == 0 else s_sb,
                        op0=mybir.AluOpType.mult,
                        op1=mybir.AluOpType.add,
                    )
                bwn = wsyn_pool.tile([O, GIKK], BF16)
                nc.scalar.sign(bwn, s_sb)
                bwn3 = bwn.rearrange("o (i t) -> o i t", t=NTAPS)
                tpP = tpsum_pool.tile([C_SH, KS * 2 * O], BF16)
                tpS = tpsum_pool.tile([C_SH, KS * O], BF16)
                tpP4 = tpP.rearrange("p (a b o) -> p a b o", b=2, o=O)
                tpS3 = tpS.rearrange("p (a o) -> p a o", o=O)
                for t in range(NTAPS):
                    ky, kx = divmod(t, KS)
                    dst = tpS3[:, kx, :] if ky == 2 else tpP4[:, kx, ky, :]
                    nc.tensor.transpose(dst, bwn3[:, :, t], identity)
                bw_my = wsyn_pool.tile([C_SH, IKK], FP8)
                nc.scalar.copy(bw_my[:, 0 : KS * 2 * O], tpP)
                nc.vector.tensor_copy(bw_my[:, KS * 2 * O : IKK], tpS)
                nc.sync.dma_start(bwg_in_t, bw_my)
                nc.gpsimd.collective_compute(
                    "AllGather",
                    mybir.AluOpType.bypass,
                    replica_groups=[list(range(N_CORES))],
                    ins=[bwg_in_t],
                    outs=[bwg_out_t],
                )
                bw_all = wsyn_pool.tile([C, IKK], FP8)
                nc.sync.dma_start(bw_all, bwg_out_t)
                bw_pair = bw_all[:, 0 : KS * 2 * O].rearrange(
                    "p (a b o) -> p a b o", b=2, o=O
                )
                bw_single = bw_all[:, KS * 2 * O : IKK].rearrange(
                    "p (a o) -> p a o", o=O
                )
                for i in range(len(X0_STRIPS)):
                    sign_x0_strip(i)
            else:
                # ---- full weight synthesis: s = M + sum_k rv_k Z_k, k-major
                # over per-chunk (per-ky) tiles so each z_k is consumed as it
                # lands ----
                GSZ = C * KS  # 384 elems per partition per chunk
                m3 = m_sb.rearrange("o (i t) -> o i t", t=NTAPS)
                z3s = [
                    z.rearrange("o (i t) -> o i t", t=NTAPS) for z in z_sbs
                ]
                s_c = [
                    wsyn_pool.tile([O, GSZ], F32, name=f"s{g}", tag=f"s{g}")
                    for g in range(NCHUNK)
                ]
                bw_c = [
                    wsyn_pool.tile([O, GSZ], BF16, name=f"bw{g}", tag=f"bw{g}")
                    for g in range(NCHUNK)
                ]
                for k in range(5):
                    for g in range(NCHUNK):
                        tsl = slice(g * KS, (g + 1) * KS)
                        nc.vector.scalar_tensor_tensor(
                            out=s_c[g].rearrange("o (i t) -> o i t", t=KS),
                            in0=z3s[k][:, :, tsl],
                            scalar=float(rv[k]),
                            in1=m3[:, :, tsl]
                            if k == 0
                            else s_c[g].rearrange("o (i t) -> o i t", t=KS),
                            op0=mybir.AluOpType.mult,
                            op1=mybir.AluOpType.add,
                        )

                # per-chunk: sign -> 3 full-width PE transposes -> pack copy.
                # fp8 psum layout: tpP[(kx, ky<2, o)] pairs, tpS[(kx, o)] the
                # ky=2 taps. bf16 layout: same split (6 + 3 taps).
                if USE_FP8:
                    bw_pair = wsyn_pool.tile([C, KS, 2, O], FP8)
                    bw_single = wsyn_pool.tile([C, KS, O], FP8)
                else:
                    bw_lhsT = wsyn_pool.tile([C, NTAPS, O], BF16)
                tpP = tpsum_pool.tile([128, KS * 2 * O], BF16)
                tpS = tpsum_pool.tile([128, KS * O], BF16)
                tpP4 = tpP.rearrange("p (a b o) -> p a b o", b=2, o=O)
                tpS3 = tpS.rearrange("p (a o) -> p a o", o=O)

                def emit_chunk(g):
                    ky = g
                    nc.scalar.sign(bw_c[g], s_c[g])
                    bw3 = bw_c[g].rearrange("o (i t) -> o i t", t=KS)
                    for kx in range(KS):
                        dst = tpS3[:, kx, :] if ky == 2 else tpP4[:, kx, ky, :]
                        nc.tensor.transpose(dst, bw3[:, :, kx], identity)

                def pack_chunk(g):
                    ky = g
                    if USE_FP8:
                        dst = (
                            bw_single.rearrange("p a o -> p (a o)")
                            if ky == 2
                            else bw_pair[:, :, ky, :]
                        )
                    else:
                        dst = bw_lhsT.rearrange("p (a t) o -> p a t o", a=KS)[
                            :, ky, :, :
                        ]
                    src = tpS if ky == 2 else tpP4[:, :, ky, :]
                    nc.vector.tensor_copy(dst, src)

                emit_chunk(0)
                emit_chunk(1)
                pack_chunk(0)
                emit_chunk(2)
                pack_chunk(1)
                pack_chunk(2)

                # x0 signs emitted after the weight path: the static
                # scheduler once ordered x0s0's sign before c2's, delaying
                # the pack that gates the conv's last two passes
                for i in range(len(X0_STRIPS)):
                    sign_x0_strip(i)

            # ---- main conv loop; next image's sign emitted before this
            # image's tiles so ACT never head-of-line blocks the sign ----
            def pair_ap(win, pair_stride):
                return bass.AP(
                    win.tensor,
                    win.offset,
                    [list(win.ap[0]), [pair_stride, 2]]
                    + [list(p) for p in win.ap[1:]],
                )

            for img in range(n_img):
                if img + 2 < n_img:
                    dma_image(img + 2)
                if img + 1 < n_img:
                    sign_image(img + 1)
                pd3 = padded[img]

                for nt in range(N_ROW_TILES):
                    y0 = nt * ROWS_PER_TILE
                    cv = cpsum_pool.tile([O, N_TILE], F32, tag="cv")
                    if USE_FP8:
                        # vertical tap pairs (ky=0,1) x 3 kx
                        for kx in range(KS):
                            win0 = pd3[:, y0 : y0 + ROWS_PER_TILE, kx : kx + W]
                            nc.tensor.matmul(
                                cv,
                                bw_pair[:, kx],
                                pair_ap(win0, WP),
                                start=(kx == 0),
                                stop=False,
                                perf_mode=mybir.MatmulPerfMode.DoubleRow,
                            )
                        if USE_HPAIR:
                            # horizontal pair: taps (2,0)+(2,1), 1B pair step
                            winh = pd3[
                                :, y0 + 2 : y0 + 2 + ROWS_PER_TILE, 0:W
                            ]
                            nc.tensor.matmul(
                                cv,
                                bw_single[:, 0:2, :],
                                pair_ap(winh, 1),
                                start=False,
                                stop=False,
                                perf_mode=mybir.MatmulPerfMode.DoubleRow,
                            )
                            win = pd3[
                                :, y0 + 2 : y0 + 2 + ROWS_PER_TILE, 2 : 2 + W
                            ]
                            nc.tensor.matmul(
                                cv, bw_single[:, 2, :], win,
                                start=False, stop=True,
                            )
                        else:
                            for kx in range(KS):
                                win = pd3[
                                    :, y0 + 2 : y0 + 2 + ROWS_PER_TILE,
                                    kx : kx + W,
                                ]
                                nc.tensor.matmul(
                                    cv, bw_single[:, kx, :], win,
                                    start=False, stop=(kx == KS - 1),
                                )
                    else:
                        t = 0
                        for ky in range(KS):
                            for kx in range(KS):
                                win = pd3[
                                    :,
                                    y0 + ky : y0 + ky + ROWS_PER_TILE,
                                    kx : kx + W,
                                ]
                                nc.tensor.matmul(
                                    cv,
                                    bw_lhsT[:, t, :],
                                    win,
                                    start=(t == 0),
                                    stop=(t == NTAPS - 1),
                                )
                                t += 1
                    ev = ev_pool.tile([O, N_TILE], F16, tag="ev")
                    ev3 = ev.rearrange("o (h w) -> o h w", w=W)
                    last_tile = (
                        img == n_img - 1 and nt == N_ROW_TILES - 1
                    )
                    if last_tile:
                        # split the final evac across DVE+ACT (separate ev
                        # tiles: same-tile WAW serializes at whole-tile
                        # granularity) and the final store across both
                        # rings: halves the end-of-kernel serial tail
                        nh = N_TILE // 2
                        hr = ROWS_PER_TILE // 2
                        evb = ev_pool.tile([O, nh], F16, tag="evb", bufs=1)
                        nc.vector.tensor_scalar_mul(
                            ev[:, 0:nh], cv[:, 0:nh], alpha_sb[:, 0:1]
                        )
                        nc.scalar.mul(
                            evb, cv[:, nh:N_TILE], alpha_sb[:, 0:1]
                        )
                        nc.scalar.dma_start(
                            out_t[img, :, y0 : y0 + hr, :], ev3[:, 0:hr, :]
                        )
                        nc.gpsimd.dma_start(
                            out_t[img, :, y0 + hr : y0 + ROWS_PER_TILE, :],
                            evb.rearrange("o (h w) -> o h w", w=W),
                        )
                    else:
                        nc.vector.tensor_scalar_mul(ev, cv, alpha_sb[:, 0:1])
                        # stores on their own queues: never head-of-line
                        # block the x loads riding the sync queue
                        dma_eng = nc.scalar if (nt % 2 == 0) else nc.gpsimd
                        dma_eng.dma_start(
                            out_t[img, :, y0 : y0 + ROWS_PER_TILE, :], ev3
                        )

    nc.compile()
    return nc


def _ensure_ntff_hook():
    """Register the axon NTFF profiling hook if the image's antenv lacks it.

    Only used when BASS_KERNEL_TRACE=1 (dev profiling); best-effort.
    """
    import sys
    import types

    try:
        import antenv

        if hasattr(antenv, "axon_hooks"):
            return
        mod = types.ModuleType("antenv.axon_hooks")
        _hook = [None]
        mod.set_axon_ntff_profile_hook = lambda h: _hook.__setitem__(0, h)
        mod.get_axon_ntff_profile_hook = lambda: _hook[0]
        sys.modules["antenv.axon_hooks"] = mod
        antenv.axon_hooks = mod
        from trn_agent_boot.trn_boot import _ntff_profile_via_ctypes

        mod.set_axon_ntff_profile_hook(
            _ntff_profile_via_ctypes("/opt/axon/libaxon_pjrt.so")
        )
    except Exception as e:  # pragma: no cover - profiling is optional
        print(f"NTFF hook registration failed ({e}); tracing disabled")


def kernel(x, Alpha, M, Z, rv):
    x = np.ascontiguousarray(np.asarray(x, dtype=np.float32))
    Alpha = np.ascontiguousarray(np.asarray(Alpha, dtype=np.float32))
    M = np.ascontiguousarray(np.asarray(M, dtype=np.float32))
    Z = np.ascontiguousarray(np.asarray(Z, dtype=np.float32))
    rv = np.asarray(rv, dtype=np.float32)

    trace = bool(int(os.environ.get("BASS_KERNEL_TRACE", "0")))
    if trace:
        _ensure_ntff_hook()

    nc = build_program(rv)

    in_maps = []
    for c in range(N_CORES):
        if USE_AG:
            m_c = np.ascontiguousarray(M[:, c * C_SH : (c + 1) * C_SH])
            z_c = np.ascontiguousarray(Z[:, :, c * C_SH : (c + 1) * C_SH])
        else:
            m_c, z_c = M, Z
        in_maps.append(
            {
                "x": np.ascontiguousarray(x[c * B_CORE : (c + 1) * B_CORE]),
                "Alpha": Alpha,
                "M": m_c,
                "Z": z_c,
            }
        )

    res = run_bass_kernel_spmd(
        nc,
        in_maps,
        core_ids=list(range(N_CORES)),
        trace=trace,
    )
    out = np.concatenate(
        [res.results[c]["out"] for c in range(N_CORES)], axis=0
    ).astype(np.float32)
    if trace:
        kernel.last_results = res
    return out
